# revision 24
# baseline (speedup 1.0000x reference)
"""Bahdanau attention on 8 Trainium2 NeuronCores.

Problem: B=8, TGT=128, SRC=256, H=512 (fp32)
    enc_proj = enc @ W_h^T            (B,S,H)
    qry_proj = q   @ W_s^T            (B,T,H)
    scores[b,t,s] = v . tanh(enc_proj[b,s] + qry_proj[b,t])
    scores masked to s < src_lengths[b], softmax over s -> attn_weights
    ctx = attn_weights @ enc; attn_out = tanh([ctx, q] @ W_out^T)
Returns (attn_out (B,T,H), attn_weights (B,T,S)).

Sharding: pure data-parallel over batch — core i owns batch element i
(weights are replicated; they are tiny next to the O(T*S*H) tanh work).

Per-core device strategy (hidden dim on the 128 SBUF partitions,
pre-transposed on the host so no on-device input transposes are needed):
  1. enc_proj^T / qry_proj^T via PE matmuls (contract over h on partitions),
     fp16 operands (fp32 PE matmuls cost 2 HW passes each; fp16 keeps a
     10-bit mantissa so the accuracy loss vs fp32 stays ~5e-4).
  2. The dominant cost is tanh on T*S*H = 16.8M elements.  The broadcast-add
     enc_proj^T[:, s] + qry_proj^T[:, t] and the tanh are split across DVE
     and ACT to balance the two engines:
       - 28/32 t's per block: DVE tensor_scalar_add (fp16 => 2x packed mode,
         per-partition fp32 scalar = qry_proj^T column), then one big ACT
         tanh over [128, 28*256] (amortizes the ~350-cycle ACT overhead).
       - 4/32 t's per block: ACT directly computes tanh(enc + q_t) via the
         activation bias path ([128, 256] each) into a separate tile so the
         two engines never write the same SBUF region.
  3. scores = v . tanh(...) contracted over the partition dim on the PE with
     a one-hot-times-v fp16 stationary matrix: lhsT[:, j] = v_chunk*(j == tt)
     so scores land directly in a [t(partitions), s(free)] layout, written
     into 32-row PSUM column-strips via tile_position=(0, 32b).  Strips
     alternate between two PSUM banks so strip b's softmax (PSUM reads) can
     overlap strip b+1's matmuls (PSUM writes).  The length mask (0/-1e30,
     host-precomputed since src_lengths is known there) is accumulated into
     each strip by a K=1 rank-1 matmul that also opens the accumulation.
  4. Per-strip masked softmax over s (t on partitions): reduce_max(negate)
     from PSUM -> exp(x - max) with per-partition bias AP + accum_out row-sum
     in one ACT instruction -> reciprocal -> tensor_scalar_mul -> DMA out,
     followed by per-strip PE transpose and per-strip ctx^T accumulation, so
     almost no serial tail remains after the last score matmul.
  5. attn_out = tanh([ctx, q] @ W_out^T) in fp32: the q-half of the
     contraction is accumulated into PSUM early (it does not depend on the
     attention), the ctx-half closes the group at the end.

Infrastructure note: this container's walrus rejects any instruction with
more than ONE semaphore sync-wait ("Too many sync wait commands").  Tile
attaches all pending waits to the consuming instruction, so we post-process
the serialized BIR JSON: extra waits are hoisted onto fresh NoOp
instructions inserted just before the consumer in the same engine's program
order (engine sequencers run in order, so wait-A-then-wait-B on separate
instructions is equivalent to one instruction waiting on both).
"""

import json
import os

import numpy as np

B, TGT, SRC, H = 8, 128, 256, 512
P = 128          # SBUF partitions
HT = H // P      # 4 h-chunks
ST = SRC // P    # 2 s-chunks
KT = (2 * H) // P  # 8 k-chunks of combined [ctx, q]
TB = 32          # t-block size (PSUM column strip)
NB = TGT // TB   # 4 t-blocks
AD = 4           # t's per (block, h-chunk) routed to the ACT-direct path

_NEG = -1.0e30

# ---------------------------------------------------------------------------
# BIR post-pass: split multi-wait instructions (see module docstring)
# ---------------------------------------------------------------------------

_wsplit_counter = [0]


def _split_multi_waits(bir_bytes):
    m = json.loads(bir_bytes)
    changed = False
    for fn in m.get("functions", []):
        for bb in fn.get("blocks", []):
            new_insts = []
            for ins in bb.get("instructions", []):
                si = ins.get("sync_info")
                waits = (si or {}).get("on_wait") or []
                if len(waits) > 1:
                    changed = True
                    for w in waits[:-1]:
                        _wsplit_counter[0] += 1
                        nop = {
                            "name": f"WSPLIT-{_wsplit_counter[0]}",
                            "engine": ins["engine"],
                            "opcode": "NoOp",
                            "ins": [],
                            "outs": [],
                            "sync_info": {"on_wait": [w], "on_update": []},
                        }
                        if "debug" in ins:
                            nop["debug"] = ins["debug"]
                        new_insts.append(nop)
                    si["on_wait"] = [waits[-1]]
                new_insts.append(ins)
            bb["instructions"] = new_insts
    return json.dumps(m).encode() if changed else bir_bytes


_patched = [False]


def _install_bir_patch():
    if _patched[0]:
        return
    import concourse.bass_utils as bass_utils
    import concourse.bass2jax as bass2jax

    orig = bass_utils.compile_bir_kernel

    def patched(bir_json, tmpdir, neff_name="file.neff"):
        if isinstance(bir_json, str):
            bir_json = bir_json.encode()
        return orig(_split_multi_waits(bir_json), tmpdir, neff_name=neff_name)

    bass_utils.compile_bir_kernel = patched
    bass2jax.compile_bir_kernel = patched
    _patched[0] = True


# ---------------------------------------------------------------------------
# Device program (built once, SPMD across the 8 cores)
# ---------------------------------------------------------------------------

_nc_cache = [None]


def _build_nc():
    if _nc_cache[0] is not None:
        return _nc_cache[0]
    import concourse.bass as bass
    import concourse.tile as tile
    from concourse import mybir

    F32 = mybir.dt.float32
    F16 = mybir.dt.float16
    AFT = mybir.ActivationFunctionType

    nc = bass.Bass()
    qT_d = nc.dram_tensor("qT", [P, HT, TGT], F16, kind="ExternalInput")
    encT_d = nc.dram_tensor("encT", [P, HT, SRC], F16, kind="ExternalInput")
    encN_d = nc.dram_tensor("encN", [P, ST, H], F16, kind="ExternalInput")
    whT_d = nc.dram_tensor("whT", [P, HT, H], F16, kind="ExternalInput")
    wsT_d = nc.dram_tensor("wsT", [P, HT, H], F16, kind="ExternalInput")
    woT_d = nc.dram_tensor("woT", [P, KT, H], F16, kind="ExternalInput")
    vmat_d = nc.dram_tensor("vmat", [P, HT, TB, TB], F16, kind="ExternalInput")
    pen_d = nc.dram_tensor("pen", [1, SRC], F16, kind="ExternalInput")
    idn_d = nc.dram_tensor("idn", [P, P], F16, kind="ExternalInput")
    attn_out_d = nc.dram_tensor("attn_out", [TGT, H], F32, kind="ExternalOutput")
    attn_w_d = nc.dram_tensor("attn_w", [TGT, SRC], F32, kind="ExternalOutput")

    with tile.TileContext(nc) as tc:
        with (
            tc.tile_pool(name="consts", bufs=1) as consts,
            tc.tile_pool(name="stage", bufs=8) as stages,
            tc.tile_pool(name="pproj", bufs=2, space="PSUM") as pp,
            tc.tile_pool(name="pscores", bufs=1, space="PSUM") as psc,
            tc.tile_pool(name="pmisc", bufs=1, space="PSUM") as pmisc,
        ):
            whT_s = consts.tile([P, HT, H], F16)
            encT_s = consts.tile([P, HT, SRC], F16)
            wsT_s = consts.tile([P, HT, H], F16)
            qT_s = consts.tile([P, HT, TGT], F16)
            vmat_s = consts.tile([P, HT, TB, TB], F16)
            pen_s = consts.tile([1, SRC], F16)
            idn_s = consts.tile([P, P], F16)
            encN_s = consts.tile([P, ST, H], F16)
            woT_s = consts.tile([P, KT, H], F16)
            # critical-path DMAs split between the two HWDGE issuers (SP and
            # ACT) — a single queue moves only ~45 GB/s, so halving each
            # tensor across two queues halves the head transfer tail.  The
            # rest rides gpsimd SWDGE.
            for k in range(HT):
                eng = nc.sync if k % 2 == 0 else nc.scalar
                eng.dma_start(encT_s[:, k, :], encT_d[:, k, :])
                eng.dma_start(whT_s[:, k, :], whT_d[:, k, :])
                eng.dma_start(wsT_s[:, k, :], wsT_d[:, k, :])
            nc.sync.dma_start(qT_s[:], qT_d[:])
            nc.sync.dma_start(vmat_s[:], vmat_d[:])
            nc.gpsimd.dma_start(pen_s[:], pen_d[:])
            nc.gpsimd.dma_start(idn_s[:], idn_d[:])
            nc.gpsimd.dma_start(encN_s[:], encN_d[:])
            nc.gpsimd.dma_start(woT_s[:], woT_d[:])

            # preload the tanh/exp ACT table set while the input DMAs run
            warm_s = consts.tile([P, 1], F32)
            nc.vector.memset(warm_s[:], 0.0)
            nc.scalar.activation(warm_s[:], warm_s[:], AFT.Tanh)
            ones_s = consts.tile([1, TB], F16)
            nc.gpsimd.memset(ones_s[:], 1.0)

            # fp16 enc_proj^T feeds the DVE broadcast-add (2x packed mode)
            # and the ACT-direct bias path; fp32 qry_proj^T feeds the scalar
            # (ptr) / bias operands which must be fp32.
            encprojT_s = consts.tile([P, HT, SRC], F16)
            qprojT_s = consts.tile([P, HT, TGT], F32)

            # proj^T[o, s/t] = sum_h W[o, h] x[s/t, h]; interleave r so the
            # first t-block's staging starts as early as possible
            b0_stageAs, b0_stageBs = [], []

            def phase1(b, r, stageAs, stageBs):
                stageB = stages.tile([P, AD, SRC], F16, tag="stageB",
                                     name=f"stageB_{b}_{r}")
                stageBs.append(stageB)
                for tt in range(AD):
                    t = b * TB + tt
                    nc.scalar.activation(
                        stageB[:, tt, :],
                        encprojT_s[:, r, :],
                        AFT.Tanh,
                        bias=qprojT_s[:, r, t:t + 1],
                        scale=1.0,
                    )
                stageA = stages.tile([P, TB - AD, SRC], F16, tag="stageA",
                                     name=f"stageA_{b}_{r}")
                stageAs.append(stageA)
                for tt in range(AD, TB):
                    t = b * TB + tt
                    nc.vector.tensor_scalar_add(
                        stageA[:, tt - AD, :],
                        encprojT_s[:, r, :],
                        qprojT_s[:, r, t:t + 1],
                    )

            for r in range(HT):
                ps = pp.tile([P, SRC], F32, tag="ps")
                for k in range(HT):
                    nc.tensor.matmul(
                        ps[:],
                        whT_s[:, k, r * P:(r + 1) * P],
                        encT_s[:, k, :],
                        start=(k == 0),
                        stop=(k == HT - 1),
                    )
                nc.scalar.copy(encprojT_s[:, r, :], ps[:])
                psq = pp.tile([P, TGT], F32, tag="ps")
                for k in range(HT):
                    nc.tensor.matmul(
                        psq[:],
                        wsT_s[:, k, r * P:(r + 1) * P],
                        qT_s[:, k, :],
                        start=(k == 0),
                        stop=(k == HT - 1),
                    )
                nc.scalar.copy(qprojT_s[:, r, :], psq[:])
                # start block 0's staging for this chunk immediately
                phase1(0, r, b0_stageAs, b0_stageBs)

            # q-half of the output contraction — independent of the attention,
            # accumulate it into PSUM while the main loop runs
            po = pmisc.tile([P, H], F32, tag="po")
            for k in range(HT, KT):
                nc.tensor.matmul(
                    po[:],
                    qT_s[:, k - HT, :],
                    woT_s[:, k, :],
                    start=(k == HT),
                    stop=False,
                    skip_group_check=True,
                )

            # scores[t, s]: 32-row column strip per t-block, strips alternate
            # between two PSUM banks so softmax reads overlap matmul writes
            scpsA = psc.tile([P, SRC], F32, tag="scpsA")
            scpsB = psc.tile([P, SRC], F32, tag="scpsB")
            attnw_s = consts.tile([P, SRC], F32)
            attnwb_s = consts.tile([P, SRC], F16)
            negmax_s = consts.tile([P, 1], F32)
            sumexp_s = consts.tile([P, 1], F32)
            rsum_s = consts.tile([P, 1], F32)
            awT_s = consts.tile([P, ST, TGT], F16)
            pc = pmisc.tile([P, HT, TGT], F32, tag="pc")

            def stage_work(b):
                scps = scpsA if b % 2 == 0 else scpsB
                strip = TB * b
                stripe = scps[strip:strip + TB, :]
                # mask opener: every row of the strip gets pen[s]
                nc.tensor.matmul(
                    stripe,
                    ones_s[:, :],
                    pen_s[:, :],
                    start=True,
                    stop=False,
                    tile_position=(0, strip),
                    skip_group_check=True,
                )
                # phase 1 (ACT-direct + DVE adds) was emitted earlier —
                # block 0's inside the projection loop, later blocks two
                # blocks ahead via phase1_block
                stageAs, stageBs = all_stages.pop(b)
                # phase 2: big tanh + score matmuls per unit.  The very last
                # unit's tanh is sub-chunked so its matmuls (which gate the
                # final softmax/output chain) trail each chunk closely.
                for r in range(HT):
                    stageA, stageB = stageAs[r], stageBs[r]
                    last_unit = (b == NB - 1 and r == HT - 1)
                    if last_unit:
                        nchunk = 4
                        csz = (TB - AD) // nchunk
                        for c in range(nchunk):
                            nc.scalar.activation(
                                stageA[:, c * csz:(c + 1) * csz, :],
                                stageA[:, c * csz:(c + 1) * csz, :],
                                AFT.Tanh)
                    else:
                        nc.scalar.activation(stageA[:], stageA[:], AFT.Tanh)
                    for tt in range(TB):
                        rhs = (stageB[:, tt, :] if tt < AD
                               else stageA[:, tt - AD, :])
                        nc.tensor.matmul(
                            stripe,
                            vmat_s[:, r, tt, :],
                            rhs,
                            start=False,
                            stop=(r == HT - 1 and tt == TB - 1),
                            tile_position=(0, strip),
                            skip_group_check=True,
                        )

            def strip_tail(b):
                scps = scpsA if b % 2 == 0 else scpsB
                strip = TB * b
                stripe = scps[strip:strip + TB, :]
                # per-strip masked softmax (strip rows = partitions)
                nm = negmax_s[strip:strip + TB, 0:1]
                se = sumexp_s[strip:strip + TB, 0:1]
                rs = rsum_s[strip:strip + TB, 0:1]
                aw = attnw_s[strip:strip + TB, :]
                nc.vector.tensor_reduce(
                    nm, stripe,
                    axis=mybir.AxisListType.X,
                    op=mybir.AluOpType.max,
                    negate=True,
                )
                nc.scalar.activation(
                    aw, stripe, AFT.Exp, bias=nm, scale=1.0, accum_out=se)
                nc.vector.reciprocal(rs, se)
                nc.vector.tensor_scalar_mul(aw, aw, rs)
                nc.sync.dma_start(attn_w_d[strip:strip + TB, :], aw)
                if b == NB - 1:
                    nc.vector.tensor_copy(attnwb_s[strip:strip + TB, :], aw)
                else:
                    nc.gpsimd.tensor_copy(attnwb_s[strip:strip + TB, :], aw)

                # per-strip transpose into aw^T columns
                for st in range(ST):
                    pt = pmisc.tile([P, TB], F16, tag="pt",
                                    name=f"pt_{b}_{st}")
                    nc.tensor.transpose(
                        pt[:, :],
                        attnwb_s[strip:strip + TB, st * P:(st + 1) * P],
                        idn_s[strip:strip + TB, strip:strip + TB],
                        tile_position=(strip, 0),
                    )
                    nc.vector.tensor_copy(awT_s[:, st, strip:strip + TB], pt[:])

                # per-strip ctx^T accumulation: pc[:, r, strip cols]
                for r in range(HT):
                    for st in range(ST):
                        nc.tensor.matmul(
                            pc[:, r, strip:strip + TB],
                            encN_s[:, st, r * P:(r + 1) * P],
                            awT_s[:, st, strip:strip + TB],
                            start=(st == 0),
                            stop=(st == ST - 1),
                            skip_group_check=True,
                        )

            # keep DVE two blocks ahead of the big-tanh consumers and lag
            # the strip tails one block so no engine waits in-order on PE
            all_stages = {0: (b0_stageAs, b0_stageBs)}

            def phase1_block(b):
                stageAs, stageBs = [], []
                for r in range(HT):
                    phase1(b, r, stageAs, stageBs)
                all_stages[b] = (stageAs, stageBs)

            phase1_block(1)
            for b in range(NB):
                stage_work(b)
                if b + 2 < NB:
                    phase1_block(b + 2)
                if b >= 1:
                    strip_tail(b - 1)
            strip_tail(NB - 1)

            # ctx^T copies interleaved with the ctx-half output matmuls
            ctxT_s = consts.tile([P, HT, TGT], F16)
            for k in range(HT):
                nc.vector.tensor_copy(ctxT_s[:, k, :], pc[:, k, :])
                nc.tensor.matmul(
                    po[:],
                    ctxT_s[:, k, :],
                    woT_s[:, k, :],
                    start=False,
                    stop=(k == HT - 1),
                    skip_group_check=True,
                )
            outt_s = consts.tile([P, H], F32)
            nc.scalar.activation(outt_s[:, 0:H // 2], po[:, 0:H // 2], AFT.Tanh)
            nc.sync.dma_start(attn_out_d[:, 0:H // 2], outt_s[:, 0:H // 2])
            nc.scalar.activation(outt_s[:, H // 2:], po[:, H // 2:], AFT.Tanh)
            nc.scalar.dma_start(attn_out_d[:, H // 2:], outt_s[:, H // 2:])

    _nc_cache[0] = nc
    return nc


# ---------------------------------------------------------------------------
# Host-side sharding / layout
# ---------------------------------------------------------------------------


def _prep_core_inputs(query, encoder_outputs, src_lengths, W_h, W_s, v, W_out):
    F16 = np.float16
    v_flat = np.asarray(v, np.float32).reshape(-1)
    assert v_flat.size == H
    # whT[p, k, c] = W_h[c(+r offset per column chunk), k*128 + p]
    whT = np.ascontiguousarray(
        np.asarray(W_h, np.float32).reshape(H, HT, P).transpose(2, 1, 0)
    ).astype(F16)
    wsT = np.ascontiguousarray(
        np.asarray(W_s, np.float32).reshape(H, HT, P).transpose(2, 1, 0)
    ).astype(F16)
    woT = np.ascontiguousarray(
        np.asarray(W_out, np.float32).reshape(H, KT, P).transpose(2, 1, 0)
    ).astype(F16)
    # vmat[p, r, tt, j] = v[r*128 + p] * (j == tt)
    v4 = v_flat.reshape(HT, P).T  # [128, 4]
    vmat = np.zeros((P, HT, TB, TB), np.float32)
    idx = np.arange(TB)
    vmat[:, :, idx, idx] = v4[:, :, None]
    vmat = np.ascontiguousarray(vmat.astype(F16))
    idn = np.eye(P, dtype=np.float32).astype(F16)

    lens = np.asarray(src_lengths).astype(np.int64).reshape(-1)
    in_maps = []
    for b in range(B):
        q = np.asarray(query[b], np.float32)
        e = np.asarray(encoder_outputs[b], np.float32)
        qT = np.ascontiguousarray(
            q.reshape(TGT, HT, P).transpose(2, 1, 0)).astype(F16)
        encT = np.ascontiguousarray(
            e.reshape(SRC, HT, P).transpose(2, 1, 0)).astype(F16)
        encN = np.ascontiguousarray(
            e.reshape(ST, P, H).transpose(1, 0, 2)).astype(F16)
        # -1e30 overflows fp16; -60000 is representable and exp(-60000-max)
        # underflows to exactly 0 the same way
        pen = np.where(np.arange(SRC) < lens[b], 0.0, -60000.0).astype(
            F16).reshape(1, SRC)
        in_maps.append({
            "qT": qT, "encT": encT, "encN": encN,
            "whT": whT, "wsT": wsT, "woT": woT,
            "vmat": vmat, "pen": np.ascontiguousarray(pen), "idn": idn,
        })
    return in_maps


LAST_RESULT = [None]


def kernel(query, encoder_outputs, src_lengths, W_h, W_s, v, W_out):
    _install_bir_patch()
    from concourse.bass_utils import run_bass_kernel_spmd

    nc = _build_nc()
    in_maps = _prep_core_inputs(
        query, encoder_outputs, src_lengths, W_h, W_s, v, W_out)

    trace = bool(int(os.environ.get("BAHDANAU_TRACE", "0")))
    kw = {}
    if trace:
        kw["trace"] = True
    res = run_bass_kernel_spmd(nc, in_maps, core_ids=list(range(B)), **kw)
    LAST_RESULT[0] = res

    attn_out = np.stack([res.results[b]["attn_out"] for b in range(B)])
    attn_w = np.stack([res.results[b]["attn_w"] for b in range(B)])
    return attn_out, attn_w
